# revision 15
# baseline (speedup 1.0000x reference)
"""Fused RNN cell on 8 Trainium2 NeuronCores.

Reference computation (fp32):
    combined   = [x, hidden]                      [B=4096, I+H=4096]
    new_hidden = tanh(combined @ W_ih^T + b_ih)   [B, H=2048]
    output     = new_hidden @ W_ho^T + b_ho       [B, O=2048]
    returns (output, new_hidden)

Strategy: data-parallel over the batch — each of the 8 cores processes 512
batch rows with replicated weights; no collectives. All operand layout
transforms (transposes into PE-friendly [K-partition, free] form) happen on
the host so every device DMA is a fat, fully contiguous transfer:

    c   [128, 32, 512]      cL[ki, ko, b]       = combined[b, ko*128+ki]
    w1  [128, 32, 16, 128]  w1L[ki, ko, hc, h]  = W_ih[hc*128+h, ko*128+ki]
    w2  [128, 16, 16, 128]  w2L[hi, ho, oc, o]  = W_ho[oc*128+o, ho*128+hi]
    b1  [128, 16]           b1L[p, hc]          = b_ih[hc*128+p]

All matmul operands are bf16 (fp32 PSUM accumulation; measured rms rel err
~3e-3, well inside the 2e-2 gate). bf16 vs fp32r costs nothing on the PE
(both 1 cycle/row) but (a) halves HBM traffic — 28 MB/core instead of
64 MB, so the kernel is never DMA-bound — and (b) enables the compiler's
automatic Fast Weight Load (4-byte dtypes are excluded from FWL), halving
the per-matmul LDWEIGHTS exposure that dominates the fp32r version.
mm1 produces nh^T [h, b] tiles in SBUF (bf16), which feed mm2 directly as
the moving operand; mm2 produces out^T [o, b]. Both outputs are stored
bf16/transposed and un-transposed + upcast on the host after the gather;
b_ho is added on the host.

Loop structure: h-chunks (and o-chunks) are processed in groups of 8, one
PSUM bank per chunk. Each inner step streams a two-ko weight slice
[128, 2, 8, 128] (plus, in the first group, two [128, 512] c-chunks) and
issues 16 accumulating matmuls, so DMA stays deep and fat while the PE
runs back-to-back matmuls. Stores ride both HWDGE rings, deferred one
group so a store waiting on compute never head-of-line blocks the load
ring. Dummy matmuls at t=0 warm the PE clock gate (HAM) and preload the
ACT tanh table during the initial DMA ramp.
"""

import numpy as np
import ml_dtypes

import concourse.bass as bass
import concourse.mybir as mybir
import concourse.tile as tile
from concourse import bacc, bass_utils

NCORES = 8
B, I, H, O = 4096, 2048, 2048, 2048
BC = B // NCORES          # 512 batch rows per core
K1 = I + H                # mm1 contraction dim, 4096
KO1 = K1 // 128           # 32 k-chunks for mm1
HC = H // 128              # 16 h-chunks
OC = O // 128              # 16 o-chunks
G = 8                     # h/o-chunks per PSUM group (8 banks)
P = 128
F32 = mybir.dt.float32
BF16 = mybir.dt.bfloat16
NP_BF16 = ml_dtypes.bfloat16
AF = mybir.ActivationFunctionType


def _build():
    nc = bacc.Bacc("TRN2", target_bir_lowering=False)

    c = nc.dram_tensor("c", [P, KO1, BC], BF16, kind="ExternalInput")
    w1 = nc.dram_tensor("w1", [P, KO1, HC, P], BF16, kind="ExternalInput")
    b1 = nc.dram_tensor("b1", [P, HC], F32, kind="ExternalInput")
    w2 = nc.dram_tensor("w2", [P, HC, OC, P], BF16, kind="ExternalInput")
    nhT = nc.dram_tensor("nhT", [H, BC], BF16, kind="ExternalOutput")
    outT = nc.dram_tensor("outT", [O, BC], BF16, kind="ExternalOutput")

    with tile.TileContext(nc) as tc:
        with tc.tile_pool(name="cpool", bufs=1) as cpool, \
             tc.tile_pool(name="wpool", bufs=10) as wpool, \
             tc.tile_pool(name="nhpool", bufs=1) as nhpool, \
             tc.tile_pool(name="opool", bufs=8) as opool, \
             tc.tile_pool(name="bpool", bufs=1) as bpool, \
             tc.tile_pool(name="ps", bufs=8, space="PSUM") as ps:

            c_sb = cpool.tile([P, KO1, BC], BF16)
            nh_sb = nhpool.tile([P, HC, BC], BF16)

            # The first two c chunks gate the first real matmul — they go
            # at the very head of the scalar ring, before b_ih (which is
            # not needed until the first group drains ~70 µs in). Keeping
            # b_ih off GpSimd SWDGE avoids 8 DMASW semaphores that would
            # lengthen the end-of-kernel drain by ~2 µs. b_ho is added on
            # the host after the gather.
            nc.scalar.dma_start(c_sb[:, 0:1], c[:, 0:1])
            nc.scalar.dma_start(c_sb[:, 1:2], c[:, 1:2])
            b1_sb = bpool.tile([P, HC], F32)
            nc.scalar.dma_start(b1_sb[:], b1[:])

            # PE warm-up: the HAM clock gate holds the PE at 1.2 GHz until
            # it has been busy ~3.4 µs. Dummy matmuls (no data deps beyond
            # one memset) keep the PE active while the first input tiles
            # stream in, so real matmuls start at 2.4 GHz.
            warm_sb = bpool.tile([P, P], mybir.dt.bfloat16)
            nc.vector.memset(warm_sb[:], 0.0)
            # Preload the ACT tanh table set (~2.7 us) during the DMA-bound
            # ramp instead of at the first group's drain.
            act_warm = bpool.tile([1, 1], F32)
            nc.scalar.activation(act_warm[:], warm_sb[:1, :1], AF.Tanh)

            # Stores are deferred one group: group g's stores are emitted
            # after group g+1's loads, so when the sync sequencer reaches
            # them the producing compute finished long ago and the ring
            # never head-of-line blocks on a store waiting for compute.
            deferred = []

            def flush_deferred():
                for fn in deferred:
                    fn()
                deferred.clear()

            # mm1: nh^T[h, b] = tanh(W_ih @ combined^T + b_ih)
            # G-sized PSUM groups ping-pong across the 8 banks: while one
            # group's banks drain through ACT, the next group accumulates
            # into the other four — group boundaries cost the PE nothing.
            for g in range(HC // G):
                psums = [ps.tile([P, BC], F32, tag="ps", name=f"ps{i}")
                         for i in range(G)]
                if g == 0:
                    # PE warm-up: HAM holds the PE at 1.2 GHz until ~3.4 us
                    # of busy time. Dummy matmuls (into the last bank this
                    # group will touch; start=True on the real group clears
                    # it) keep the PE active while the first tiles stream
                    # in, so real matmuls run at 2.4 GHz from the start.
                    # ~18 × ~110 ns ≈ 2 µs of PE busy — ends right as the
                    # first (finely split) c/w tiles land (~9.4 µs). The
                    # first few real matmuls run below full clock (HAM
                    # reaches 2.4 GHz ~3 µs after the PE goes busy) but
                    # starting 2.3 µs earlier wins overall; ending the
                    # warmups early would risk an idle gap resetting HAM.
                    for _ in range(18):
                        nc.tensor.matmul(
                            psums[G - 1][:, :P], lhsT=warm_sb[:],
                            rhs=warm_sb[:],
                            start=True, stop=True, skip_group_check=True,
                        )
                for ko0 in range(0, KO1, 2):
                    if g == 0 and ko0 > 0:
                        # c rides the scalar HWDGE ring so the sync ring
                        # carries only weights: with both streams on one
                        # ring the first matmul of every group-0 step
                        # stalls ~160 ns on its c chunk queued behind the
                        # previous step's w slice. (ko 0-1 were issued at
                        # the top, ahead of b_ih.)
                        nc.scalar.dma_start(
                            c_sb[:, ko0:ko0 + 2], c[:, ko0:ko0 + 2])
                    if g == 0 and ko0 == 0:
                        # Step 0 gates the first real matmul: split the
                        # weight slice into three separate tiles (64 KB /
                        # 192 KB / 256 KB) so matmul i only waits for the
                        # piece it actually reads — the first one lands
                        # ~2.5 µs before the full 512 KB slice would.
                        w0a = wpool.tile([P, 1, 2, P], BF16, name="w0a")
                        nc.sync.dma_start(w0a[:], w1[:, 0:1, 0:2])
                        w0b = wpool.tile([P, 1, G - 2, P], BF16, name="w0b")
                        nc.sync.dma_start(w0b[:], w1[:, 0:1, 2:G])
                        w0c = wpool.tile([P, 1, G, P], BF16, name="w0c")
                        nc.sync.dma_start(w0c[:], w1[:, 1:2, 0:G])
                        for i in range(G):
                            nc.tensor.matmul(
                                psums[i][:],
                                lhsT=(w0a[:, 0, i] if i < 2 else
                                      w0b[:, 0, i - 2]),
                                rhs=c_sb[:, 0],
                                start=True, stop=False,
                            )
                        for i in range(G):
                            nc.tensor.matmul(
                                psums[i][:], lhsT=w0c[:, 0, i],
                                rhs=c_sb[:, 1],
                                start=False, stop=False,
                            )
                        continue
                    w1_sb = wpool.tile([P, 2, G, P], BF16, tag="w")
                    nc.sync.dma_start(
                        w1_sb[:], w1[:, ko0:ko0 + 2, g * G:(g + 1) * G])
                    for kk in range(2):
                        for i in range(G):
                            nc.tensor.matmul(
                                psums[i][:],
                                lhsT=w1_sb[:, kk, i],
                                rhs=c_sb[:, ko0 + kk],
                                start=(ko0 + kk == 0),
                                stop=(ko0 + kk == KO1 - 1),
                            )
                flush_deferred()
                for i in range(G):
                    hc = g * G + i
                    nc.scalar.activation(
                        nh_sb[:, hc], psums[i][:], AF.Tanh,
                        bias=b1_sb[:, hc:hc + 1],
                    )
                    deferred.append(
                        lambda hc=hc: nc.sync.dma_start(
                            nhT[hc * P:(hc + 1) * P, :], nh_sb[:, hc])
                    )

            # mm2: out^T[o, b] = W_ho @ nh^T + b_ho
            # Groups of [8, 4, 4] o-chunks: the two trailing 4-groups
            # ping-pong through the 8 PSUM banks (no boundary stall) and
            # the final drain chain is half as long, shortening the tail.
            # (Smaller tail groups measured slower: one w2 trigger per 4
            # matmuls makes the sync sequencer the bottleneck.)
            for g0, gsz in ((0, 8), (8, 4), (12, 4)):
                psums = [ps.tile([P, BC], F32, tag="ps", name=f"ps{i}")
                         for i in range(gsz)]
                for ho0 in range(0, HC, 2):
                    w2_sb = wpool.tile([P, 2, G, P], BF16, tag="w", name="w2_sb")[:, :, :gsz]
                    nc.sync.dma_start(
                        w2_sb[:], w2[:, ho0:ho0 + 2, g0:g0 + gsz])
                    for kk in range(2):
                        for i in range(gsz):
                            nc.tensor.matmul(
                                psums[i][:],
                                lhsT=w2_sb[:, kk, i],
                                rhs=nh_sb[:, ho0 + kk],
                                start=(ho0 + kk == 0),
                                stop=(ho0 + kk == HC - 1),
                            )
                flush_deferred()
                # Evict PSUM through both DVE and ACT in parallel (raw
                # copies; b_ho is added on the host). ACT-evicted tiles
                # store via the ACT HWDGE ring right behind their copy;
                # DVE-evicted tiles store via the sync ring, deferred one
                # group so the ring never waits on the copy.
                for i in range(gsz):
                    oc = g0 + i
                    o_sb = opool.tile([P, BC], BF16, tag="osb")
                    if i % 2:
                        nc.scalar.activation(o_sb[:], psums[i][:], AF.Copy)
                        nc.scalar.dma_start(
                            outT[oc * P:(oc + 1) * P, :], o_sb[:])
                    else:
                        nc.vector.tensor_copy(o_sb[:], psums[i][:])
                        deferred.append(
                            lambda oc=oc, o_sb=o_sb: nc.sync.dma_start(
                                outT[oc * P:(oc + 1) * P, :], o_sb[:])
                        )
            flush_deferred()

    nc.compile()
    return nc


def _shard_inputs(x, hidden, W_ih, b_ih, W_ho, b_ho):
    combined = np.concatenate([x, hidden], axis=1)  # [B, K1]
    w1L = np.ascontiguousarray(
        W_ih.reshape(HC, P, KO1, P).transpose(3, 2, 0, 1).astype(NP_BF16)
    )  # [ki, ko, hc, h]
    w2L = np.ascontiguousarray(
        W_ho.reshape(OC, P, HC, P).transpose(3, 2, 0, 1).astype(NP_BF16)
    )  # [hi, ho, oc, o]
    b1L = np.ascontiguousarray(b_ih.reshape(HC, P).T)
    in_maps = []
    for cix in range(NCORES):
        cc = combined[cix * BC:(cix + 1) * BC]  # [BC, K1]
        cL = np.ascontiguousarray(
            cc.reshape(BC, KO1, P).transpose(2, 1, 0).astype(NP_BF16))
        in_maps.append(
            {"c": cL, "w1": w1L, "b1": b1L, "w2": w2L}
        )
    return in_maps


def _run(in_maps, **kwargs):
    nc = _build()
    return bass_utils.run_bass_kernel_spmd(
        nc, in_maps, core_ids=list(range(NCORES)), **kwargs
    )


def kernel(x, hidden, W_ih, b_ih, W_ho, b_ho):
    x = np.asarray(x, dtype=np.float32)
    hidden = np.asarray(hidden, dtype=np.float32)
    W_ih = np.asarray(W_ih, dtype=np.float32)
    b_ih = np.asarray(b_ih, dtype=np.float32)
    W_ho = np.asarray(W_ho, dtype=np.float32)
    b_ho = np.asarray(b_ho, dtype=np.float32)

    in_maps = _shard_inputs(x, hidden, W_ih, b_ih, W_ho, b_ho)
    res = _run(in_maps)
    output = np.concatenate(
        [r["outT"].astype(np.float32).T for r in res.results], axis=0) + b_ho
    new_hidden = np.concatenate(
        [r["nhT"].astype(np.float32).T for r in res.results], axis=0)
    return output, new_hidden


# revision 18
# speedup vs baseline: 1.0017x; 1.0017x over previous
"""Fused RNN cell on 8 Trainium2 NeuronCores.

Reference computation (fp32):
    combined   = [x, hidden]                      [B=4096, I+H=4096]
    new_hidden = tanh(combined @ W_ih^T + b_ih)   [B, H=2048]
    output     = new_hidden @ W_ho^T + b_ho       [B, O=2048]
    returns (output, new_hidden)

Strategy: data-parallel over the batch — each of the 8 cores processes 512
batch rows with replicated weights; no collectives. All operand layout
transforms (transposes into PE-friendly [K-partition, free] form) happen on
the host so every device DMA is a fat, fully contiguous transfer:

    c   [128, 32, 512]      cL[ki, ko, b]       = combined[b, ko*128+ki]
    w1  [128, 32, 16, 128]  w1L[ki, ko, hc, h]  = W_ih[hc*128+h, ko*128+ki]
    w2  [128, 16, 16, 128]  w2L[hi, ho, oc, o]  = W_ho[oc*128+o, ho*128+hi]
    b1  [128, 16]           b1L[p, hc]          = b_ih[hc*128+p]

All matmul operands are bf16 (fp32 PSUM accumulation; measured rms rel err
~3e-3, well inside the 2e-2 gate). bf16 vs fp32r costs nothing on the PE
(both 1 cycle/row) but (a) halves HBM traffic — 28 MB/core instead of
64 MB, so the kernel is never DMA-bound — and (b) enables the compiler's
automatic Fast Weight Load (4-byte dtypes are excluded from FWL), halving
the per-matmul LDWEIGHTS exposure that dominates the fp32r version.
mm1 produces nh^T [h, b] tiles in SBUF (bf16), which feed mm2 directly as
the moving operand; mm2 produces out^T [o, b]. Both outputs are stored
bf16/transposed and un-transposed + upcast on the host after the gather;
b_ho is added on the host.

Loop structure: h-chunks (and o-chunks) are processed in groups of 8, one
PSUM bank per chunk. Each inner step streams a two-ko weight slice
[128, 2, 8, 128] (plus, in the first group, two [128, 512] c-chunks) and
issues 16 accumulating matmuls, so DMA stays deep and fat while the PE
runs back-to-back matmuls. Stores ride both HWDGE rings, deferred one
group so a store waiting on compute never head-of-line blocks the load
ring. Dummy matmuls at t=0 warm the PE clock gate (HAM) and preload the
ACT tanh table during the initial DMA ramp.
"""

import numpy as np
import ml_dtypes

import concourse.bass as bass
import concourse.mybir as mybir
import concourse.tile as tile
from concourse import bacc, bass_utils

NCORES = 8
B, I, H, O = 4096, 2048, 2048, 2048
BC = B // NCORES          # 512 batch rows per core
K1 = I + H                # mm1 contraction dim, 4096
KO1 = K1 // 128           # 32 k-chunks for mm1
HC = H // 128              # 16 h-chunks
OC = O // 128              # 16 o-chunks
G = 8                     # h/o-chunks per PSUM group (8 banks)
P = 128
F32 = mybir.dt.float32
BF16 = mybir.dt.bfloat16
NP_BF16 = ml_dtypes.bfloat16
AF = mybir.ActivationFunctionType


def _build():
    nc = bacc.Bacc("TRN2", target_bir_lowering=False)

    c = nc.dram_tensor("c", [P, KO1, BC], BF16, kind="ExternalInput")
    w1 = nc.dram_tensor("w1", [P, KO1, HC, P], BF16, kind="ExternalInput")
    b1 = nc.dram_tensor("b1", [P, HC], F32, kind="ExternalInput")
    w2 = nc.dram_tensor("w2", [P, HC, OC, P], BF16, kind="ExternalInput")
    nhT = nc.dram_tensor("nhT", [H, BC], BF16, kind="ExternalOutput")
    outT = nc.dram_tensor("outT", [O, BC], BF16, kind="ExternalOutput")

    with tile.TileContext(nc) as tc:
        with tc.tile_pool(name="cpool", bufs=1) as cpool, \
             tc.tile_pool(name="wpool", bufs=10) as wpool, \
             tc.tile_pool(name="nhpool", bufs=1) as nhpool, \
             tc.tile_pool(name="opool", bufs=8) as opool, \
             tc.tile_pool(name="bpool", bufs=1) as bpool, \
             tc.tile_pool(name="ps", bufs=8, space="PSUM") as ps:

            c_sb = cpool.tile([P, KO1, BC], BF16)
            nh_sb = nhpool.tile([P, HC, BC], BF16)

            # The first two c chunks gate the first real matmul — they go
            # at the very head of the scalar ring, before b_ih (which is
            # not needed until the first group drains ~70 µs in). Keeping
            # b_ih off GpSimd SWDGE avoids 8 DMASW semaphores that would
            # lengthen the end-of-kernel drain by ~2 µs. b_ho is added on
            # the host after the gather.
            nc.scalar.dma_start(c_sb[:, 0:1], c[:, 0:1])
            nc.scalar.dma_start(c_sb[:, 1:2], c[:, 1:2])
            b1_sb = bpool.tile([P, HC], F32)
            nc.scalar.dma_start(b1_sb[:], b1[:])

            # PE warm-up: the HAM clock gate holds the PE at 1.2 GHz (or
            # lower) until it has been busy ~3 µs, and an idle gap resets
            # the ramp. Dummy matmuls keep the PE active from the earliest
            # possible instant until the first input tiles stream in, so
            # real matmuls start at 2.4 GHz. The warmup operand is read
            # UNINITIALIZED (no memset) — its numerical content is
            # irrelevant (start=True on the first real matmul clears the
            # bank) and a memset would delay the first warmup ~0.5 µs.
            warm_sb = bpool.tile([P, P], mybir.dt.bfloat16)
            # (memset on GpSimd — the only engine with nothing else to do
            # at t=0 — so the first warmup matmul isn't delayed.)
            nc.gpsimd.memset(warm_sb[:], 0.0)
            # Preload the ACT tanh table set (~2.7 us) during the DMA-bound
            # ramp instead of at the first group's drain.
            act_warm = bpool.tile([1, 1], F32)
            nc.scalar.activation(act_warm[:], warm_sb[:1, :1], AF.Tanh)

            # Stores are deferred one group: group g's stores are emitted
            # after group g+1's loads, so when the sync sequencer reaches
            # them the producing compute finished long ago and the ring
            # never head-of-line blocks on a store waiting for compute.
            deferred = []

            def flush_deferred():
                for fn in deferred:
                    fn()
                deferred.clear()

            # mm1: nh^T[h, b] = tanh(W_ih @ combined^T + b_ih)
            # G-sized PSUM groups ping-pong across the 8 banks: while one
            # group's banks drain through ACT, the next group accumulates
            # into the other four — group boundaries cost the PE nothing.
            for g in range(HC // G):
                psums = [ps.tile([P, BC], F32, tag="ps", name=f"ps{i}")
                         for i in range(G)]
                if g == 0:
                    # PE warm-up: HAM holds the PE at 1.2 GHz until ~3.4 us
                    # of busy time. Dummy matmuls (into the last bank this
                    # group will touch; start=True on the real group clears
                    # it) keep the PE active while the first tiles stream
                    # in, so real matmuls run at 2.4 GHz from the start.
                    # ~30 × ~107 ns ≈ 3.2 µs of PE busy — bridges with NO
                    # idle gap to the first real matmul (data lands ~10.8
                    # µs) while covering the ~3 µs HAM ramp; a gap resets
                    # the clock ramp and costs ~3 µs of 1.2 GHz matmuls.
                    for _ in range(30):
                        nc.tensor.matmul(
                            psums[G - 1][:, :P], lhsT=warm_sb[:],
                            rhs=warm_sb[:],
                            start=True, stop=True, skip_group_check=True,
                        )
                for ko0 in range(0, KO1, 2):
                    if g == 0 and ko0 > 0:
                        # c rides the scalar HWDGE ring so the sync ring
                        # carries only weights: with both streams on one
                        # ring the first matmul of every group-0 step
                        # stalls ~160 ns on its c chunk queued behind the
                        # previous step's w slice. (ko 0-1 were issued at
                        # the top, ahead of b_ih.)
                        nc.scalar.dma_start(
                            c_sb[:, ko0:ko0 + 2], c[:, ko0:ko0 + 2])
                    if g == 0 and ko0 == 0:
                        # Step 0 gates the first real matmul: split the
                        # weight slice into three separate tiles (64 KB /
                        # 192 KB / 256 KB) so matmul i only waits for the
                        # piece it actually reads — the first one lands
                        # ~2.5 µs before the full 512 KB slice would.
                        w0a = wpool.tile([P, 1, 2, P], BF16, name="w0a")
                        nc.sync.dma_start(w0a[:], w1[:, 0:1, 0:2])
                        w0b = wpool.tile([P, 1, G - 2, P], BF16, name="w0b")
                        nc.sync.dma_start(w0b[:], w1[:, 0:1, 2:G])
                        w0c = wpool.tile([P, 1, G, P], BF16, name="w0c")
                        nc.sync.dma_start(w0c[:], w1[:, 1:2, 0:G])
                        for i in range(G):
                            nc.tensor.matmul(
                                psums[i][:],
                                lhsT=(w0a[:, 0, i] if i < 2 else
                                      w0b[:, 0, i - 2]),
                                rhs=c_sb[:, 0],
                                start=True, stop=False,
                            )
                        for i in range(G):
                            nc.tensor.matmul(
                                psums[i][:], lhsT=w0c[:, 0, i],
                                rhs=c_sb[:, 1],
                                start=False, stop=False,
                            )
                        continue
                    w1_sb = wpool.tile([P, 2, G, P], BF16, tag="w")
                    nc.sync.dma_start(
                        w1_sb[:], w1[:, ko0:ko0 + 2, g * G:(g + 1) * G])
                    for kk in range(2):
                        for i in range(G):
                            nc.tensor.matmul(
                                psums[i][:],
                                lhsT=w1_sb[:, kk, i],
                                rhs=c_sb[:, ko0 + kk],
                                start=(ko0 + kk == 0),
                                stop=(ko0 + kk == KO1 - 1),
                            )
                flush_deferred()
                for i in range(G):
                    hc = g * G + i
                    nc.scalar.activation(
                        nh_sb[:, hc], psums[i][:], AF.Tanh,
                        bias=b1_sb[:, hc:hc + 1],
                    )
                    deferred.append(
                        lambda hc=hc: nc.sync.dma_start(
                            nhT[hc * P:(hc + 1) * P, :], nh_sb[:, hc])
                    )

            # mm2: out^T[o, b] = W_ho @ nh^T + b_ho
            # Groups of [8, 4, 4] o-chunks: the two trailing 4-groups
            # ping-pong through the 8 PSUM banks (no boundary stall) and
            # the final drain chain is half as long, shortening the tail.
            # (Smaller tail groups measured slower: one w2 trigger per 4
            # matmuls makes the sync sequencer the bottleneck.)
            for g0, gsz in ((0, 8), (8, 4), (12, 4)):
                psums = [ps.tile([P, BC], F32, tag="ps", name=f"ps{i}")
                         for i in range(gsz)]
                for ho0 in range(0, HC, 2):
                    w2_sb = wpool.tile([P, 2, G, P], BF16, tag="w", name="w2_sb")[:, :, :gsz]
                    nc.sync.dma_start(
                        w2_sb[:], w2[:, ho0:ho0 + 2, g0:g0 + gsz])
                    for kk in range(2):
                        for i in range(gsz):
                            nc.tensor.matmul(
                                psums[i][:],
                                lhsT=w2_sb[:, kk, i],
                                rhs=nh_sb[:, ho0 + kk],
                                start=(ho0 + kk == 0),
                                stop=(ho0 + kk == HC - 1),
                            )
                flush_deferred()
                # Evict PSUM through both DVE and ACT in parallel (raw
                # copies; b_ho is added on the host). ACT-evicted tiles
                # store via the ACT HWDGE ring right behind their copy;
                # DVE-evicted tiles store via the sync ring, deferred one
                # group so the ring never waits on the copy.
                for i in range(gsz):
                    oc = g0 + i
                    o_sb = opool.tile([P, BC], BF16, tag="osb")
                    if i % 2:
                        nc.scalar.activation(o_sb[:], psums[i][:], AF.Copy)
                        nc.scalar.dma_start(
                            outT[oc * P:(oc + 1) * P, :], o_sb[:])
                    else:
                        nc.vector.tensor_copy(o_sb[:], psums[i][:])
                        deferred.append(
                            lambda oc=oc, o_sb=o_sb: nc.sync.dma_start(
                                outT[oc * P:(oc + 1) * P, :], o_sb[:])
                        )
            flush_deferred()

    nc.compile()
    return nc


def _shard_inputs(x, hidden, W_ih, b_ih, W_ho, b_ho):
    combined = np.concatenate([x, hidden], axis=1)  # [B, K1]
    w1L = np.ascontiguousarray(
        W_ih.reshape(HC, P, KO1, P).transpose(3, 2, 0, 1).astype(NP_BF16)
    )  # [ki, ko, hc, h]
    w2L = np.ascontiguousarray(
        W_ho.reshape(OC, P, HC, P).transpose(3, 2, 0, 1).astype(NP_BF16)
    )  # [hi, ho, oc, o]
    b1L = np.ascontiguousarray(b_ih.reshape(HC, P).T)
    in_maps = []
    for cix in range(NCORES):
        cc = combined[cix * BC:(cix + 1) * BC]  # [BC, K1]
        cL = np.ascontiguousarray(
            cc.reshape(BC, KO1, P).transpose(2, 1, 0).astype(NP_BF16))
        in_maps.append(
            {"c": cL, "w1": w1L, "b1": b1L, "w2": w2L}
        )
    return in_maps


def _run(in_maps, **kwargs):
    nc = _build()
    return bass_utils.run_bass_kernel_spmd(
        nc, in_maps, core_ids=list(range(NCORES)), **kwargs
    )


def kernel(x, hidden, W_ih, b_ih, W_ho, b_ho):
    x = np.asarray(x, dtype=np.float32)
    hidden = np.asarray(hidden, dtype=np.float32)
    W_ih = np.asarray(W_ih, dtype=np.float32)
    b_ih = np.asarray(b_ih, dtype=np.float32)
    W_ho = np.asarray(W_ho, dtype=np.float32)
    b_ho = np.asarray(b_ho, dtype=np.float32)

    in_maps = _shard_inputs(x, hidden, W_ih, b_ih, W_ho, b_ho)
    res = _run(in_maps)
    output = np.concatenate(
        [r["outT"].astype(np.float32).T for r in res.results], axis=0) + b_ho
    new_hidden = np.concatenate(
        [r["nhT"].astype(np.float32).T for r in res.results], axis=0)
    return output, new_hidden
